# revision 11
# baseline (speedup 1.0000x reference)
"""Trainium2 Bass kernel for nn_ContinuousConvolutionBlock (gnn_message_passing).

Strategy (per sharding hint: partition points across 8 cores; each core owns its
queries' scatter-reduce and tap-GEMM; filter + dense weights replicated):

Host side (index plumbing / input marshalling only — zero FLOPs):
  - qry_idx is sorted; queries are grouped into 8-query blocks, blocks paired
    into 128-edge-slot "chunks" (two-pointer bin packing, ~3% padding).
  - Consecutive block ranges are assigned to the 8 cores; per-core per-slot
    payload arrays (pos[src], pos[qry], feats[src] (bf16), local query id) are
    marshalled on host and DMA'd in dense [128 x NCH x k] layout.

Device side (all FLOP-bearing compute):
  - Geometry: ball->cube volume-preserving map (DVE arithmetic + ACT
    sqrt/arctan/sign/abs) on UNSCALED relative coords (map is linear in scale,
    folded into the grid transform), then trilinear corner weights via the
    hat function w[ax] = relu(1 - |g - ax|), duplicated x2 along the tap axis
    (bf16 "dup-pair" packing) so downstream DVE ops hit the 2x packed mode.
  - Scatter-reduce as factored matmul per 128-slot chunk: with
    R[slot,(ax,c)] = w4x (x) feats  (bf16) and
    L[slot,(t,hq)]  = zy (x) Qoh    (bf16, t=(az,ay), hq = query-in-chunk),
    PE computes A^T[(ax,c),(t,hq)] = R^T @ L per chunk into PSUM.
  - PSUM->SBUF cast-copies re-arrange A into at[(ax,c), (t, chunk, hq)] bf16
    so each tap-GEMM rhs slice is fully contiguous.
  - Tap-GEMM: out^T += G_t^T @ at[:, t-slice], accumulated over 16 t in PSUM.
    G is the filter regrouped on host (pure relayout, replicated to all cores).
  - Dense branch: out_dense^T = dense_w^T @ feats^T (+bias via ACT) on PE.
  Outputs are produced transposed ([64, nq]); host transposes/reorders back.
"""
import sys
import os
sys.path.insert(0, '/opt/trn_rl_repo')
import numpy as np
import ml_dtypes

N = 30000
CIN = 32
COUT = 64
KS = 4
EXTENT = 0.08
NCORES = 8
NBLK = N // 8  # 3750 eight-query blocks

BF16 = ml_dtypes.bfloat16

_COMPILED = {}

# tuning knobs
SEG_GROUPS = [2, 6, 7]       # geometry segment sizes (in 16-chunk groups)
R_ON_POOL = lambda g: g % 3 != 0   # which groups' R builds go to Pool
COPY_DVE = lambda g, q: (g % 3 == 0 and q == 3)  # which quad copies on DVE


# ----------------------------------------------------------------------------
# Host planning
# ----------------------------------------------------------------------------
def _plan(qry_idx):
    deg = np.bincount(qry_idx, minlength=N)
    bsz = deg.reshape(NBLK, 8).sum(1)
    bstart = np.concatenate([[0], np.cumsum(bsz)]).astype(np.int64)
    per = [NBLK // NCORES + (1 if c < NBLK % NCORES else 0) for c in range(NCORES)]
    b0 = np.concatenate([[0], np.cumsum(per)]).astype(np.int64)
    plans = []
    for c in range(NCORES):
        blocks = list(range(b0[c], b0[c + 1]))
        asc = sorted(blocks, key=lambda b: bsz[b])
        chunks = []
        lo, hi = 0, len(asc) - 1
        while lo <= hi:
            if lo == hi:
                chunks.append((asc[hi], None)); break
            if bsz[asc[hi]] + bsz[asc[lo]] <= 128:
                chunks.append((asc[hi], asc[lo])); hi -= 1; lo += 1
            else:
                chunks.append((asc[hi], None)); hi -= 1
        plans.append(dict(blocks=blocks, chunks=chunks, q0=int(8 * b0[c]),
                          nq=int(8 * (b0[c + 1] - b0[c]))))
    return plans, bstart, bsz


def _pack_core(plan_c, bstart, pos, feats, qry_idx, src_idx, NCHP):
    """Build per-slot payload arrays in [128, NCHP, k] layout."""
    possrc = np.zeros((128, NCHP, 3), np.float32)
    posqry = np.zeros((128, NCHP, 3), np.float32)
    fsrc = np.zeros((128, NCHP, CIN), BF16)
    qlocf = np.full((128, NCHP, 2), -1.0, BF16)
    for ci, (bA, bB) in enumerate(plan_c['chunks']):
        s = 0
        for half, b in enumerate((bA, bB)):
            if b is None:
                continue
            e0, e1 = int(bstart[b]), int(bstart[b + 1])
            n = e1 - e0
            sl = slice(s, s + n)
            possrc[sl, ci, :] = pos[src_idx[e0:e1]]
            posqry[sl, ci, :] = pos[qry_idx[e0:e1]]
            fsrc[sl, ci, :] = feats[src_idx[e0:e1]].astype(BF16)
            ql = ((qry_idx[e0:e1] - 8 * b) + 8 * half).astype(BF16)
            qlocf[sl, ci, 0] = ql
            qlocf[sl, ci, 1] = ql
            s += n
    return possrc, posqry, fsrc, qlocf


# ----------------------------------------------------------------------------
# Device kernel
# ----------------------------------------------------------------------------
def _build_bass(NCHP, NQ):
    import concourse.bass as bass
    import concourse.tile as tile
    from concourse import bacc, mybir
    from concourse.bass import AP

    f32 = mybir.dt.float32
    bf16 = mybir.dt.bfloat16
    i32 = mybir.dt.int32
    ALU = mybir.AluOpType
    ACT = mybir.ActivationFunctionType
    EPS = 1e-12
    F4PI = float(4.0 / np.pi)
    GSCL = float(1.5 * 2.0 / EXTENT)  # grid scale folded: g = GSCL*m + 1.5

    nc = bacc.Bacc("TRN2", target_bir_lowering=False, debug=False)

    possrc = nc.dram_tensor("possrc", (128, NCHP, 3), f32, kind="ExternalInput")
    posqry = nc.dram_tensor("posqry", (128, NCHP, 3), f32, kind="ExternalInput")
    fsrc = nc.dram_tensor("fsrc", (128, NCHP, CIN), bf16, kind="ExternalInput")
    qlocf = nc.dram_tensor("qlocf", (128, NCHP, 2), bf16, kind="ExternalInput")
    g2 = nc.dram_tensor("g2", (128, 16 * 64), bf16, kind="ExternalInput")
    featsT = nc.dram_tensor("featsT", (CIN, NQ), bf16, kind="ExternalInput")
    denw = nc.dram_tensor("denw", (CIN, COUT), bf16, kind="ExternalInput")
    denb = nc.dram_tensor("denb", (COUT, 1), f32, kind="ExternalInput")

    outconvT = nc.dram_tensor("outconvT", (COUT, NQ), f32, kind="ExternalOutput")
    outdenseT = nc.dram_tensor("outdenseT", (COUT, NQ), f32, kind="ExternalOutput")

    W = NCHP
    NGRP = NCHP // 16
    assert sum(SEG_GROUPS) == NGRP, (SEG_GROUPS, NGRP)
    segs = []
    g0 = 0
    for n in SEG_GROUPS:
        segs.append((g0, g0 + n))
        g0 += n

    with tile.TileContext(nc) as tc:
        with tc.tile_pool(name="inp", bufs=1) as inp, \
             tc.tile_pool(name="geo", bufs=1) as geo, \
             tc.tile_pool(name="tmp", bufs=1) as tmp, \
             tc.tile_pool(name="lp", bufs=3) as lpool, \
             tc.tile_pool(name="rp", bufs=4) as rpool, \
             tc.tile_pool(name="at", bufs=2) as atp, \
             tc.tile_pool(name="outp", bufs=4) as outp, \
             tc.tile_pool(name="ps1", bufs=3, space="PSUM") as ps1, \
             tc.tile_pool(name="ps2", bufs=2, space="PSUM") as ps2:

            # ---------------- input DMAs (priority order) ----------------
            t_ps = inp.tile([128, W, 3], f32)
            t_pq = inp.tile([128, W, 3], f32)
            t_f = inp.tile([128, W, CIN], bf16)
            t_ql = inp.tile([128, W, 2], bf16)
            t_g2 = inp.tile([128, 16 * 64], bf16)
            t_ftT = inp.tile([CIN, NQ], bf16)
            t_dw = inp.tile([CIN, COUT], bf16)
            t_db = inp.tile([COUT, 1], f32)
            nc.sync.dma_start(t_ps[:], possrc[:])
            nc.sync.dma_start(t_pq[:], posqry[:])
            nc.sync.dma_start(t_ql[:], qlocf[:])
            nc.sync.dma_start(t_ftT[:], featsT[:])
            nc.sync.dma_start(t_dw[:], denw[:])
            nc.sync.dma_start(t_db[:], denb[:])
            nc.sync.dma_start(t_f[:], fsrc[:])
            nc.sync.dma_start(t_g2[:], g2[:])

            # iota constants
            io16i = tmp.tile([128, 16], i32)
            nc.gpsimd.iota(io16i[:], pattern=[[1, 16]], base=0, channel_multiplier=0)
            io16b = geo.tile([128, 16], bf16)
            nc.vector.tensor_copy(io16b[:], io16i[:])
            # c4m = [0,0,1,1,2,2,3,3] - 1.5  (dup-pair tap offsets)
            c4di = tmp.tile([128, 8], i32)
            nc.gpsimd.iota(c4di[:], pattern=[[1, 4], [0, 2]], base=0,
                           channel_multiplier=0)
            c4m = geo.tile([128, 8], f32)
            nc.vector.tensor_copy(c4m[:], c4di[:])
            nc.vector.tensor_scalar(c4m[:], c4m[:], 1.5, None, op0=ALU.subtract)

            # ---------------- dense branch (bf16 matmul, runs first) --------
            NSEG = (NQ + 511) // 512
            for s in range(NSEG):
                j0 = s * 512
                j1 = min(NQ, j0 + 512)
                pd = ps2.tile([COUT, 512], f32, space="PSUM", tag="po")
                nc.tensor.matmul(
                    out=pd[:, 0:j1 - j0],
                    lhsT=t_dw[:],
                    rhs=t_ftT[:, j0:j1],
                    start=True, stop=True)
                odt = outp.tile([COUT, 512], f32, tag="odst")
                db = t_db[:, 0:1]
                nc.scalar.activation(odt[:, 0:j1 - j0], pd[:, 0:j1 - j0],
                                     ACT.Identity, bias=db, scale=1.0)
                nc.sync.dma_start(outdenseT[:, j0:j1], odt[:, 0:j1 - j0])

            # ---------------- temp tile machinery ----------------
            _tn = [0]
            _free_tags = []
            _tag_of = {}
            _seq = [0]

            def T(shape, dt_=f32):
                key = tuple(shape) + (dt_,)
                for i, (tg, k) in enumerate(_free_tags):
                    if k == key:
                        _free_tags.pop(i)
                        break
                else:
                    _tn[0] += 1
                    tg = f"t{_tn[0]}"
                _seq[0] += 1
                t = tmp.tile(list(shape), dt_, name=f"{tg}_u{_seq[0]}", tag=tg)
                _tag_of[id(t)] = (tg, key)
                return t

            def F(*ts):
                for t in ts:
                    _free_tags.append(_tag_of.pop(id(t)))

            TT = nc.vector.tensor_tensor
            TS = nc.vector.tensor_scalar
            STT = nc.vector.scalar_tensor_tensor

            # delayed tap-GEMM state for PE software pipelining
            pend = []

            def flush_tap():
                if not pend:
                    return
                at_t, gg = pend.pop(0)
                po = ps2.tile([COUT, 512], f32, space="PSUM", tag="po")
                for t in range(16):
                    nc.tensor.matmul(
                        out=po[:, 0:256],
                        lhsT=t_g2[:, t * 64:(t + 1) * 64],
                        rhs=at_t[:, t * 256:(t + 1) * 256],
                        start=(t == 0), stop=(t == 15))
                ost = outp.tile([COUT, 256], f32, tag="ocst")
                nc.scalar.copy(ost[:], po[:, 0:256])
                nc.sync.dma_start(outconvT[:, gg * 256:(gg + 1) * 256], ost[:])

            for (g_lo, g_hi) in segs:
                c0 = g_lo * 16
                Wh = (g_hi - g_lo) * 16

                # ---------------- geometry on [128, Wh] ----------------
                rs = T((128, Wh, 3))
                TT(out=rs[:], in0=t_ps[:, c0:c0 + Wh, :],
                   in1=t_pq[:, c0:c0 + Wh, :], op=ALU.subtract)
                z = rs[:, :, 2]

                sq3 = T((128, Wh, 3))
                TT(out=sq3[:], in0=rs[:], in1=rs[:], op=ALU.mult)
                x2, y2, z2 = sq3[:, :, 0], sq3[:, :, 1], sq3[:, :, 2]
                xy2 = T((128, Wh))
                TT(out=xy2[:], in0=x2, in1=y2, op=ALU.add)

                sq = T((128, Wh))
                TT(out=sq[:], in0=xy2[:], in1=z2, op=ALU.add)
                norm = T((128, Wh))
                nc.scalar.activation(norm[:], sq[:], ACT.Sqrt)
                F(sq)
                nxy = T((128, Wh))
                nc.scalar.activation(nxy[:], xy2[:], ACT.Sqrt)

                pole = T((128, Wh))
                STT(out=pole[:], in0=z2, scalar=1.25, in1=xy2[:],
                    op0=ALU.mult, op1=ALU.is_gt)
                F(sq3, xy2)

                azn = T((128, Wh))
                nc.scalar.activation(azn[:], z, ACT.Abs)
                den1 = T((128, Wh))
                STT(out=den1[:], in0=azn[:], scalar=EPS, in1=norm[:],
                    op0=ALU.add, op1=ALU.add)
                rd1 = T((128, Wh))
                nc.vector.reciprocal_approx_fast(rd1[:], den1[:])
                t1s = T((128, Wh))
                STT(out=t1s[:], in0=norm[:], scalar=3.0, in1=rd1[:],
                    op0=ALU.mult, op1=ALU.mult)
                s1 = T((128, Wh))
                nc.scalar.activation(s1[:], t1s[:], ACT.Sqrt)
                F(azn, den1, rd1, t1s)

                den2 = T((128, Wh))
                TS(den2[:], nxy[:], EPS, None, op0=ALU.add)
                rd2 = T((128, Wh))
                nc.vector.reciprocal_approx_fast(rd2[:], den2[:])
                s2 = T((128, Wh))
                TT(out=s2[:], in0=norm[:], in1=rd2[:], op=ALU.mult)
                F(nxy, den2, rd2)

                d12 = T((128, Wh))
                TT(out=d12[:], in0=s1[:], in1=s2[:], op=ALU.subtract)
                pw = T((128, Wh))
                TT(out=pw[:], in0=pole[:], in1=d12[:], op=ALU.mult)
                wq = T((128, Wh))
                TT(out=wq[:], in0=s2[:], in1=pw[:], op=ALU.add)
                F(s1, s2, d12, pw)

                # xc, yc in one paired op
                xcyc = T((128, Wh, 2))
                TT(out=xcyc[:], in0=rs[:, :, 0:2],
                   in1=AP(wq.tensor, wq[:].offset, [wq[:].ap[0], [1, Wh], [0, 2]]),
                   op=ALU.mult)

                sgz = T((128, Wh))
                nc.scalar.activation(sgz[:], z, ACT.Sign)
                zcp = T((128, Wh))
                TT(out=zcp[:], in0=sgz[:], in1=norm[:], op=ALU.mult)
                zce = T((128, Wh))
                TS(zce[:], z, 1.5, None, op0=ALU.mult)
                dz = T((128, Wh))
                TT(out=dz[:], in0=zcp[:], in1=zce[:], op=ALU.subtract)
                pz = T((128, Wh))
                TT(out=pz[:], in0=pole[:], in1=dz[:], op=ALU.mult)
                zc = T((128, Wh))
                TT(out=zc[:], in0=zce[:], in1=pz[:], op=ALU.add)
                F(sgz, zcp, zce, dz, pz, pole, norm, rs, wq)

                # cylinder -> cube
                c2 = T((128, Wh, 2))
                TT(out=c2[:], in0=xcyc[:], in1=xcyc[:], op=ALU.mult)
                sqxy = T((128, Wh))
                TT(out=sqxy[:], in0=c2[:, :, 0], in1=c2[:, :, 1], op=ALU.add)
                nrm = T((128, Wh))
                nc.scalar.activation(nrm[:], sqxy[:], ACT.Sqrt)
                F(c2, sqxy)

                axy = T((128, Wh, 2))
                nc.scalar.activation(axy[:], xcyc[:], ACT.Abs)
                abr = T((128, Wh))
                TT(out=abr[:], in0=axy[:, :, 1], in1=axy[:, :, 0], op=ALU.is_le)

                mm2 = T((128, Wh, 2))
                TS(mm2[:], axy[:], EPS, None, op0=ALU.is_lt)
                sf2 = T((128, Wh, 2))
                TT(out=sf2[:], in0=xcyc[:], in1=mm2[:], op=ALU.add)
                F(axy, mm2)
                rcp2 = T((128, Wh, 2))
                nc.vector.reciprocal_approx_fast(rcp2[:], sf2[:])
                rat2 = T((128, Wh, 2))
                TT(out=rat2[:], in0=xcyc[:],
                   in1=AP(rcp2.tensor, rcp2[:].offset + 1,
                          [rcp2[:].ap[0], [2, Wh], [-1, 2]]),
                   op=ALU.mult)
                at12 = T((128, Wh, 2))
                nc.scalar.activation(at12[:], rat2[:], ACT.Arctan)
                sg2 = T((128, Wh, 2))
                nc.scalar.activation(sg2[:], xcyc[:], ACT.Sign)
                F(sf2, rcp2, rat2)

                # Q = [tmpa, tmpb, xoe, yoe]
                Q = T((128, Wh, 4))
                TT(out=Q[:, :, 0:2], in0=sg2[:],
                   in1=AP(nrm.tensor, nrm[:].offset,
                          [nrm[:].ap[0], [1, Wh], [0, 2]]),
                   op=ALU.mult)
                STT(out=Q[:, :, 2:4], in0=at12[:], scalar=F4PI,
                    in1=AP(Q.tensor, Q[:].offset + 1,
                           [Q[:].ap[0], [4, Wh], [-1, 2]]),
                    op0=ALU.mult, op1=ALU.mult)
                F(sg2, nrm, at12, xcyc)

                # xo = xoe + abr*(tmpa-xoe); yo = tmpb + abr*(yoe-tmpb)
                a2 = AP(Q.tensor, Q[:].offset, [Q[:].ap[0], [4, Wh], [3, 2]])
                b2 = AP(Q.tensor, Q[:].offset + 2, [Q[:].ap[0], [4, Wh], [-1, 2]])
                d2 = T((128, Wh, 2))
                TT(out=d2[:], in0=a2, in1=b2, op=ALU.subtract)
                md = T((128, Wh, 2))
                TT(out=md[:], in0=d2[:],
                   in1=AP(abr.tensor, abr[:].offset,
                          [abr[:].ap[0], [1, Wh], [0, 2]]),
                   op=ALU.mult)
                xoyo = T((128, Wh, 2))
                TT(out=xoyo[:], in0=b2, in1=md[:], op=ALU.add)
                F(d2, md, abr)

                # ---------------- hat weights, dup-pair packed bf16 --------
                def hat_w4(m_ap, w4_t):
                    d = T((128, Wh, 8))
                    STT(out=d[:],
                        in0=m_ap,
                        scalar=GSCL,
                        in1=AP(c4m.tensor, c4m[:].offset,
                               [c4m[:].ap[0], [0, Wh], [1, 8]]),
                        op0=ALU.mult, op1=ALU.subtract)
                    a = T((128, Wh, 8))
                    nc.scalar.activation(a[:], d[:], ACT.Abs)
                    nc.scalar.activation(w4_t[:], a[:], ACT.Relu,
                                         bias=1.0, scale=-1.0)
                    F(d, a)

                w4x2 = geo.tile([128, Wh, 8], bf16, tag=f"w4x2_{g_lo}")
                w4y2 = T((128, Wh, 8), bf16)
                w4z2 = T((128, Wh, 8), bf16)
                hat_w4(AP(xoyo.tensor, xoyo[:].offset,
                          [xoyo[:].ap[0], [2, Wh], [0, 8]]), w4x2)
                hat_w4(AP(xoyo.tensor, xoyo[:].offset + 1,
                          [xoyo[:].ap[0], [2, Wh], [0, 8]]), w4y2)
                hat_w4(AP(zc.tensor, zc[:].offset,
                          [zc[:].ap[0], [1, Wh], [0, 8]]), w4z2)
                F(xoyo, zc)

                # zy2[slot, az*8 + ay*2 + r] = w4z[az]*w4y[ay]  (bf16 2x)
                zy2 = geo.tile([128, Wh, 32], bf16, tag=f"zy2_{g_lo}")
                for az in range(4):
                    zslc = w4z2[:, :, 2 * az:2 * az + 2]
                    TT(out=AP(zy2.tensor, zy2[:].offset + az * 8,
                              [zy2[:].ap[0], [32, Wh], [1, 8]]),
                       in0=AP(w4z2.tensor, zslc.offset,
                              [zslc.ap[0], [8, Wh], [0, 4], [1, 2]]),
                       in1=AP(w4y2.tensor, w4y2[:].offset,
                              [w4y2[:].ap[0], [8, Wh], [1, 8]]),
                       op=ALU.mult)
                F(w4y2, w4z2)

                # qoh[slot, hq] = (qloc == hq)  (bf16 2x via dup'd qloc)
                qoh = geo.tile([128, Wh, 16], bf16, tag=f"qoh_{g_lo}")
                qslc = t_ql[:, c0:c0 + Wh, :]
                TT(out=qoh[:],
                   in0=AP(t_ql.tensor, qslc.offset,
                          [qslc.ap[0], [2, Wh], [0, 8], [1, 2]]),
                   in1=AP(io16b.tensor, io16b[:].offset,
                          [io16b[:].ap[0], [0, Wh], [1, 16]]),
                   op=ALU.is_equal)

                # ---------------- per-group builds + matmuls ----------------
                for g in range(g_lo, g_hi):
                    gl = g - g_lo
                    L = lpool.tile([128, 4096], bf16, tag="L")
                    TT(out=AP(L.tensor, L[:].offset,
                              [L[:].ap[0], [16, 256], [1, 16]]),
                       in0=AP(zy2.tensor, zy2[:].offset + gl * 16 * 32,
                              [zy2[:].ap[0], [2, 256], [0, 8], [1, 2]]),
                       in1=AP(qoh.tensor, qoh[:].offset + gl * 16 * 16,
                              [qoh[:].ap[0], [16, 16], [0, 16], [1, 16]]),
                       op=ALU.mult)
                    R = rpool.tile([128, 2048], bf16, tag="R")
                    r_eng = (nc.gpsimd.tensor_tensor if R_ON_POOL(g)
                             else nc.vector.tensor_tensor)
                    r_eng(out=AP(R.tensor, R[:].offset,
                                 [R[:].ap[0], [32, 64], [1, 32]]),
                          in0=AP(w4x2.tensor, w4x2[:].offset + gl * 16 * 8,
                                 [w4x2[:].ap[0], [2, 64], [0, 16], [1, 2]]),
                          in1=AP(t_f.tensor, t_f[:].offset + g * 16 * CIN,
                                 [t_f[:].ap[0], [32, 16], [0, 4], [1, 32]]),
                          op=ALU.mult)

                    at_t = atp.tile([128, 4096], bf16, tag="at")
                    for q in range(4):
                        ps_t = ps1.tile([128, 1024], f32, space="PSUM", tag="s1")
                        for k in range(4):
                            ci = q * 4 + k
                            nc.tensor.matmul(
                                out=ps_t[:, k * 256:(k + 1) * 256],
                                lhsT=R[:, ci * 128:(ci + 1) * 128],
                                rhs=L[:, ci * 256:(ci + 1) * 256],
                                start=True, stop=True)
                        dst = AP(at_t.tensor, at_t[:].offset + q * 4 * 16,
                                 [at_t[:].ap[0], [16, 4], [256, 16], [1, 16]])
                        src = AP(ps_t.tensor, ps_t[:].offset,
                                 [ps_t[:].ap[0], [256, 4], [16, 16], [1, 16]])
                        if COPY_DVE(g, q):
                            nc.vector.tensor_copy(dst, src)
                        else:
                            nc.scalar.copy(dst, src)
                    pend.append((at_t, g))
                    if len(pend) > 1:
                        flush_tap()
            while pend:
                flush_tap()

    nc.compile()
    return nc


# ----------------------------------------------------------------------------
# Host-side input prep (shared by kernel() and test.py's profile path)
# ----------------------------------------------------------------------------
def _prepare(feats, pos, filt, dense_w, dense_b, src_idx, qry_idx):
    feats = np.ascontiguousarray(np.asarray(feats, np.float32))
    pos = np.ascontiguousarray(np.asarray(pos, np.float32))
    filt = np.asarray(filt, np.float32)
    dense_w = np.asarray(dense_w, np.float32)
    dense_b = np.asarray(dense_b, np.float32)
    src_idx = np.asarray(src_idx).astype(np.int64)
    qry_idx = np.asarray(qry_idx).astype(np.int64)

    plans, bstart, bsz = _plan(qry_idx)
    NCH = max(len(p['chunks']) for p in plans)
    NCHP = ((NCH + 15) // 16) * 16
    NQ = NCHP * 16

    # filter regroup: G2[ax*32+c, t*64+o] = filt[az, ay, ax, c, o], t = az*4+ay
    G2 = np.zeros((128, 16 * 64), np.float32)
    for az in range(4):
        for ay in range(4):
            t = az * 4 + ay
            for ax in range(4):
                G2[ax * 32:(ax + 1) * 32, t * 64:(t + 1) * 64] = filt[az, ay, ax]
    G2 = G2.astype(BF16)

    in_maps = []
    for c, p in enumerate(plans):
        possrc, posqry, fsrc, qlocf = _pack_core(p, bstart, pos, feats,
                                                 qry_idx, src_idx, NCHP)
        ftT = np.zeros((CIN, NQ), BF16)
        ftT[:, 0:p['nq']] = feats[p['q0']:p['q0'] + p['nq']].T.astype(BF16)
        in_maps.append({
            "possrc": possrc, "posqry": posqry, "fsrc": fsrc, "qlocf": qlocf,
            "g2": G2, "featsT": ftT, "denw": dense_w.astype(BF16),
            "denb": dense_b.reshape(COUT, 1).astype(np.float32),
        })
    return plans, in_maps, NCHP, NQ


# ----------------------------------------------------------------------------
# Entry point
# ----------------------------------------------------------------------------
def kernel(feats, pos, filt, dense_w, dense_b, src_idx, qry_idx):
    from concourse.bass_utils import run_bass_kernel_spmd

    plans, in_maps, NCHP, NQ = _prepare(feats, pos, filt, dense_w, dense_b,
                                        src_idx, qry_idx)

    key = (NCHP, NQ)
    if key not in _COMPILED:
        _COMPILED[key] = _build_bass(NCHP, NQ)
    nc = _COMPILED[key]

    res = run_bass_kernel_spmd(nc, in_maps, core_ids=list(range(NCORES)))

    ans_conv = np.zeros((N, COUT), np.float32)
    ans_dense = np.zeros((N, COUT), np.float32)
    for c, p in enumerate(plans):
        outT = res.results[c]["outconvT"]
        for ci, (bA, bB) in enumerate(p['chunks']):
            for half, b in enumerate((bA, bB)):
                if b is None:
                    continue
                cols = ci * 16 + half * 8
                ans_conv[8 * b:8 * b + 8] = outT[:, cols:cols + 8].T
        dT = res.results[c]["outdenseT"]
        ans_dense[p['q0']:p['q0'] + p['nq']] = dT[:, 0:p['nq']].T
    return ans_conv, ans_dense
